# revision 1
# baseline (speedup 1.0000x reference)
"""Trainium2 Bass kernel for nn_Attention (dense transformer spatial attention).

Reference computation (per batch b of 4):
  X = x[b] reshaped [256, 4096]                      (4096 = 64*64 pixels)
  QKV = w_qkv @ X -> [384, 4096]; q,k,v = split(QKV) each [128, 4096]
  per head h (4 heads x 32 dims): sim = (q_h*scale)^T k_h   [4096, 4096]
  attn = softmax(sim, axis=-1); out_h = attn @ v_h^T        [4096, 32]
  H = concat_heads -> [128, 4096]; out = w_out @ H + b_out  [256, 4096]

Sharding: 8 cores = (batch b in 0..3) x (query half qh in 0..1).
Each core gets full X_b (for K/V) plus its query-half slice, computes
attention output for its 2048 queries over all 4096 keys, and the final
projection. Gather on host is pure concatenation + transpose (device emits
[i, o] layout).

Device algorithm (per core), designed around engine rooflines (the kernel
is ScalarE-bound: 33.5M softmax exp evaluations per core at 1 elem/lane/
cycle is the hard floor, ~240us; PE/DVE/DMA work hides underneath):
  - Matmuls in float32r (FP22 multiply, 1-pass full-rate on the PE); the
    AV stage uses bf16 operands (the fused f32r weight-load path cannot
    target col-offset PSUM, and mixed f32r/bf16 operands are rejected).
  - sim is computed TRANSPOSED: simT[j, i] = sum_d k[d,j] q[d,i], via
    row-packed K=32 matmuls (one per head, tile_position=(32h,0)), so no
    transposes of the big attention matrix are ever needed. Each step is
    split into two head-pair halves over SEPARATE psum tiles (simA/simB)
    so the next step's matmuls overlap the other half's exp activation.
  - softmax: max-subtraction is skipped (|scale*sim| <~ 20 always, exp is
    safe in f32); scale is folded into the ScalarE exp activation.
  - denominator: V^T is augmented with a ones column then zero-padded to
    M=64, so the AV matmul computes sum_j exp*v AND sum_j exp in one pass.
  - AV: out^T[d_aug, i] = sum_j vTaug[j, d_aug] expT[j, i], accumulated
    over j tiles in PSUM; heads col-packed in pairs at tile_position
    (0,0)/(0,64). Every matmul accumulation group owns whole PSUM banks
    (has_written zeroing is 2KB-region granular, NOT per element).
  - normalization + projection emit outT [i, o] so the per-query softmax
    denominator is applied with plain DVE ops; host transposes back.
  - PSUM budget (8 banks): sim halves 2x[128,1024] = 4 banks, AV pair
    accumulators 2x[128,1024] = 4 banks. Every accumulation group owns
    whole banks; pool-slot WAR is tile-granular, which is why the sim
    halves are separate tiles rather than halves of one tile.
"""

import numpy as np

import concourse.bacc as bacc
import concourse.bass as bass
import concourse.mybir as mybir
import concourse.tile as tile
from concourse.bass_utils import run_bass_kernel_spmd


F32 = mybir.dt.float32
F32R = mybir.dt.float32r
BF16 = mybir.dt.bfloat16

HEADS = 4
DH = 32                      # dim per head
C = 256                      # input channels
NJ = 4096                    # keys per batch (64*64)
NI = 2048                    # queries per core (half of 4096)
JT = 128                     # j tile (partition dim of simT)
NJT = NJ // JT               # 32 j tiles
NT = 512                     # i tile for sim/exp/AV matmuls
CHUNK = 1024                 # i chunk held in AV psum accumulators
NCHUNK = NI // CHUNK         # 2
SCALE = float(DH) ** -0.5
BW = NJ + NI + 3 * 128       # blob256 width


def build_kernel(dbg=False):
    nc = bacc.Bacc("TRN2", debug=False, num_devices=8)

    # blob256 columns: [wqkvT (384) | xq (2048) | x (4096)] - weights and
    # query slice first so the q projection can start while x still streams
    # blob128 columns: [woutA (256) | woutB (256) | bias replicated (256)]
    blob256_d = nc.dram_tensor("blob256", [C, BW], F32R, kind="ExternalInput").ap()
    blob128_d = nc.dram_tensor("blob128", [128, 3 * C], F32R, kind="ExternalInput").ap()
    out_d = nc.dram_tensor("out_t", [NI, C], F32, kind="ExternalOutput").ap()
    # DRAM bounce buffer for partition-broadcasting softmax reciprocals
    # (SBUF->SBUF DMA cannot have a zero partition step on the source).
    rscr_d = nc.dram_tensor("rbscratch", [NCHUNK, 4, CHUNK], F32).ap()
    if dbg:
        dumps = {n: nc.dram_tensor("dump_" + n, s, d, kind="ExternalOutput").ap()
                 for n, s, d in [
                     ("q", [128, NI], F32), ("k", [128, NJ], F32),
                     ("vT", [128, NJT * HEADS * 64], BF16),
                     ("h1", [128, NI], F32), ("h2", [128, NI], F32),
                     ("rb1", [128, CHUNK], F32), ("rb2", [128, CHUNK], F32)]}

    with tile.TileContext(nc) as tc:
        with (
            tc.tile_pool(name="singles", bufs=1) as singles,
            tc.tile_pool(name="expp", bufs=3) as expp,
            tc.tile_pool(name="exp6", bufs=14) as exp6,
            tc.tile_pool(name="outp", bufs=6) as outp,
            tc.tile_pool(name="psim", bufs=1, space="PSUM") as psim,
            tc.tile_pool(name="pav", bufs=2, space="PSUM") as pav,
        ):
            # ---- resident SBUF tensors ----
            blob_sb = singles.tile([128, 2, BW], F32R)    # w|xq|x, 2 c-tiles
            w_sb = blob_sb[:, :, 0:3 * 128]
            xq_sb = blob_sb[:, :, 3 * 128:3 * 128 + NI]
            x_sb = blob_sb[:, :, 3 * 128 + NI:BW]
            b128_sb = singles.tile([128, 3 * C], F32R)
            woutA_sb = b128_sb[:, 0:C]
            woutB_sb = b128_sb[:, C:2 * C]
            bias_sb = b128_sb[:, 2 * C:3 * C]
            q_sb = singles.tile([128, NI], F32R)          # q rows = 4h x 32d
            k_sb = singles.tile([128, NJ], F32R)
            # vT padded to 64 cols: [v dims (32) | ones (1) | zeros (31)]
            # (M=64 keeps the (0,64) col-tiled AV matmul ISA-valid; matmul
            # cost is N-bound so the padding is free)
            # bf16: the fused f32r weight-load path cannot target col-offset
            # PSUM (tile_position (0,64)); bf16 uses the normal LDWEIGHTS path
            vT_sb = singles.tile([128, NJT, HEADS, 64], BF16)
            h1_sb = singles.tile([128, NI], F32R)         # heads 0/1 at rows 0-31/64-95
            h2_sb = singles.tile([128, NI], F32R)         # heads 2/3 at rows 0-31/64-95
            rb1_sb = singles.tile([128, CHUNK], F32)     # 1/denom bcast for h1 rows
            rb2_sb = singles.tile([128, CHUNK], F32)
            dstg_sb = singles.tile([128, 2 * CHUNK], F32)  # epilogue staging

            # single SWDGE queue -> one semaphore for all initial loads
            # (HWDGE round-robins queues and early matmuls then exceed the
            # per-instruction sync-wait slot limit)
            # One contiguous DMA per c-tile: every matmul then transitively
            # depends on exactly ONE DMA (walrus allows only ~2 semaphore
            # waits per instruction, so the wait sets must stay tiny).
            W0 = 3 * 128 + NI          # w + xq prefix
            for ct in range(2):
                nc.sync.dma_start(out=blob_sb[:, ct, 0:W0],
                                  in_=blob256_d[ct * 128:(ct + 1) * 128, 0:W0])
            for ct in range(2):
                for xh in range(4):
                    lo = W0 + xh * (NJ // 4)
                    nc.sync.dma_start(out=blob_sb[:, ct, lo:lo + NJ // 4],
                                      in_=blob256_d[ct * 128:(ct + 1) * 128,
                                                    lo:lo + NJ // 4])
            nc.sync.dma_start(out=b128_sb, in_=blob128_d)

            nc.vector.memset(h1_sb[:, :].bitcast(F32), 0.0)  # unused rows stay 0
            nc.vector.memset(h2_sb[:, :].bitcast(F32), 0.0)
            nc.vector.memset(vT_sb, 0.0)                # zero padding
            nc.vector.memset(vT_sb[:, :, :, DH], 1.0)   # ones column
            nc.vector.memset(rb1_sb, 0.0)
            nc.vector.memset(rb2_sb, 0.0)

            # trigger the ScalarE exp table load (~2.7us) during phase 1
            # instead of at the first real softmax activation
            warm = singles.tile([1, 1], F32)
            nc.vector.memset(warm, 0.0)
            nc.scalar.activation(warm, warm, mybir.ActivationFunctionType.Exp)

            # ---- phase 1: qkv projections ----
            # ordered to match DMA arrival: q needs only w+xq (first DMAs),
            # k-half0/vT(0-15) need x-half0, the rest needs x-half1
            for qg in range(2):
                psq = psim.tile([128, NI // 2], F32, tag="simA" if qg == 0 else "simB")
                for nt in range(2):
                    col = qg * 1024 + nt * 512
                    for ct in range(2):
                        nc.tensor.matmul(
                            psq[:, nt * 512:(nt + 1) * 512],
                            lhsT=w_sb[:, ct, 0:128],
                            rhs=xq_sb[:, ct, col:col + 512],
                            start=(ct == 0), stop=(ct == 1),
                        )
                nc.vector.tensor_copy(q_sb[:, qg * 1024:(qg + 1) * 1024], psq)

            def emit_k_half(half):
                for kg in range(2):
                    psk = psim.tile([128, NI // 2], F32,
                                    tag="simA" if kg == 0 else "simB")
                    for nt in range(2):
                        col = half * NI + kg * 1024 + nt * 512
                        for ct in range(2):
                            nc.tensor.matmul(
                                psk[:, nt * 512:(nt + 1) * 512],
                                lhsT=w_sb[:, ct, 128:256],
                                rhs=x_sb[:, ct, col:col + 512],
                                start=(ct == 0), stop=(ct == 1),
                            )
                    nc.vector.tensor_copy(
                        k_sb[:, half * NI + kg * 1024:half * NI + (kg + 1) * 1024],
                        psk)

            def emit_vt_range(lo, hi):
                # vT[j, vc] = sum_c x[c, j] wv[vc, c], one [128, 128] tile per jt
                for jt in range(lo, hi):
                    psv = pav.tile([128, 128], F32, tag="av")
                    for ct in range(2):
                        nc.tensor.matmul(
                            psv,
                            lhsT=x_sb[:, ct, jt * JT:(jt + 1) * JT],
                            rhs=w_sb[:, ct, 256:384],
                            start=(ct == 0), stop=(ct == 1),
                        )
                    nc.vector.tensor_copy(vT_sb[:, jt, :, 0:DH], psv)

            emit_k_half(0)
            emit_vt_range(0, NJT // 2)
            emit_k_half(1)
            emit_vt_range(NJT // 2, NJT)

            # ---- phase 2: attention main loop ----
            for chunk in range(NCHUNK):
                co = chunk * CHUNK
                avA = pav.tile([128, CHUNK], F32, tag="av")  # heads 0 @0-32, 1 @64-96
                avB = pav.tile([128, CHUNK], F32, tag="av")  # heads 2 @0-32, 3 @64-96
                def emit_av(ex, jt, nt):
                    for h in range(HEADS):
                        av = avA if h < 2 else avB
                        po = 64 * (h % 2)
                        nc.tensor.matmul(
                            av[po:po + 64, nt * NT:(nt + 1) * NT],
                            lhsT=vT_sb[:, jt, h, :],
                            rhs=ex[:, h * NT:(h + 1) * NT],
                            start=(jt == 0), stop=(jt == NJT - 1),
                            tile_position=(0, po),
                            skip_group_check=True,
                        )

                # AV for step s is emitted after sim for step s+1 so the PE
                # unblocks the ScalarE exp (the critical path) first
                # the quad/exp pair is split in two halves over SEPARATE
                # psum tiles (tags simA/simB): the next step's heads-0/1
                # matmuls overlap the current heads-2/3 exp, so the PE never
                # sits on the ScalarE critical path
                pending = None
                for jt in range(NJT):
                    for nt in range(CHUNK // NT):
                        io = co + nt * NT
                        ex = exp6.tile([128, HEADS * NT], BF16, tag="exp")
                        for grp, tag in ((0, "simA"), (1, "simB")):
                            sim = psim.tile([128, 2 * NT], F32, tag=tag)
                            for hi in range(2):
                                h = grp * 2 + hi
                                nc.tensor.matmul(
                                    sim[:, hi * NT:(hi + 1) * NT],
                                    lhsT=k_sb[h * DH:(h + 1) * DH,
                                              jt * JT:(jt + 1) * JT],
                                    rhs=q_sb[h * DH:(h + 1) * DH, io:io + NT],
                                    start=True, stop=True,
                                    tile_position=(h * DH, 0),
                                )
                            nc.scalar.activation(
                                ex[:, grp * 2 * NT:(grp + 1) * 2 * NT], sim,
                                mybir.ActivationFunctionType.Exp, scale=SCALE)
                        if pending is not None:
                            emit_av(*pending)
                        pending = (ex, jt, nt)
                emit_av(*pending)

                # softmax denominators (rows 32 & 96 of the av tiles):
                # bounce the 4 rows through DRAM to pack them into [4, CHUNK]
                # (reciprocal is free-dim bound: one packed call is 4x
                # cheaper than four [1, CHUNK] calls), then reciprocal,
                # bounce back, and partition-broadcast over each head's rows.
                # reciprocal is free-dim bound (8 cyc/elem): repack the
                # 4xCHUNK denominators as [32, CHUNK/8] via the DRAM bounce
                # so the divide runs 8x wider across partitions
                den32 = expp.tile([32, CHUNK // 8], F32, tag="rc")
                rc32 = expp.tile([32, CHUNK // 8], F32, tag="rc")
                dstg = dstg_sb
                for idx, av in enumerate((avA, avB)):
                    for pi, po in enumerate((0, 64)):
                        h4 = idx * 2 + pi
                        cs = idx * CHUNK
                        # DMA cannot read PSUM: stage the row via DVE
                        # (same partition, pair-tiles split by free offset)
                        nc.vector.tensor_copy(dstg[po + DH:po + DH + 1, cs:cs + CHUNK],
                                              av[po + DH:po + DH + 1, :])
                        nc.sync.dma_start(out=rscr_d[chunk, h4, :],
                                          in_=dstg[po + DH:po + DH + 1, cs:cs + CHUNK])
                packed = rscr_d[chunk, :, :].rearrange(
                    "a (b c) -> (a b) c", c=CHUNK // 8)
                nc.sync.dma_start(out=den32, in_=packed)
                nc.vector.reciprocal(out=rc32, in_=den32)
                nc.sync.dma_start(out=packed, in_=rc32)
                for idx, rb in enumerate((rb1_sb, rb2_sb)):
                    for pi, po in enumerate((0, 64)):
                        h4 = idx * 2 + pi
                        nc.sync.dma_start(
                            out=rb[po:po + DH, :],
                            in_=rscr_d[chunk, h4:h4 + 1, :].to_broadcast((DH, CHUNK)),
                        )
                # fused normalize + PSUM->SBUF copy
                for (av, rb, hsb) in ((avA, rb1_sb, h1_sb), (avB, rb2_sb, h2_sb)):
                    for po in (0, 64):
                        nc.vector.tensor_tensor(
                            out=hsb[po:po + DH, co:co + CHUNK],
                            in0=av[po:po + DH, :],
                            in1=rb[po:po + DH, :],
                            op=mybir.AluOpType.mult,
                        )

                # ---- output projection for this chunk ----
                for it in range(CHUNK // 128):
                    io = co + it * 128
                    pj = pav.tile([128, C], F32, tag="av")
                    nc.tensor.matmul(pj, lhsT=h1_sb[:, io:io + 128],
                                     rhs=woutA_sb, start=True, stop=False)
                    nc.tensor.matmul(pj, lhsT=h2_sb[:, io:io + 128],
                                     rhs=woutB_sb, start=False, stop=True)
                    ot = outp.tile([128, C], F32, tag="out")
                    nc.vector.tensor_tensor(out=ot, in0=pj, in1=bias_sb,
                                            op=mybir.AluOpType.add)
                    nc.sync.dma_start(out=out_d[io:io + 128, :], in_=ot)

            if dbg:
                nc.sync.dma_start(out=dumps["q"], in_=q_sb[:, :].bitcast(F32))
                nc.sync.dma_start(out=dumps["k"], in_=k_sb[:, :].bitcast(F32))
                nc.sync.dma_start(out=dumps["vT"],
                                  in_=vT_sb[:, :, :, :].rearrange("p a b c -> p (a b c)"))
                nc.sync.dma_start(out=dumps["h1"], in_=h1_sb[:, :].bitcast(F32))
                nc.sync.dma_start(out=dumps["h2"], in_=h2_sb[:, :].bitcast(F32))
                nc.sync.dma_start(out=dumps["rb1"], in_=rb1_sb)
                nc.sync.dma_start(out=dumps["rb2"], in_=rb2_sb)

    nc.compile()
    return nc


_NC = None


def _get_nc():
    global _NC
    if _NC is None:
        _NC = build_kernel()
    return _NC


def make_in_maps(x, w_qkv, w_out, b_out):
    x = np.ascontiguousarray(np.asarray(x, dtype=np.float32))
    w_qkv = np.asarray(w_qkv, dtype=np.float32)
    w_out = np.asarray(w_out, dtype=np.float32)
    b_out = np.asarray(b_out, dtype=np.float32)

    wqkvT = w_qkv.T                                       # [256, 384]
    woutT = w_out.T                                       # [128 hidden, 256]
    # projection weights permuted to the AV psum partition layout:
    # A: rows 0-31 = head0, rows 64-95 = head1; B: head2, head3; rest zero
    woutA = np.zeros((128, C), np.float32)
    woutB = np.zeros((128, C), np.float32)
    woutA[0:32] = woutT[0:32]
    woutA[64:96] = woutT[32:64]
    woutB[0:32] = woutT[64:96]
    woutB[64:96] = woutT[96:128]
    blob128 = np.ascontiguousarray(
        np.concatenate([woutA, woutB,
                        np.broadcast_to(b_out[None, :], (128, C))], axis=1))

    in_maps = []
    for core in range(8):
        b, qh = divmod(core, 2)
        xb = x[b].reshape(C, NJ)
        xqb = xb[:, qh * NI:(qh + 1) * NI]
        blob256 = np.ascontiguousarray(
            np.concatenate([wqkvT, xqb, xb], axis=1))
        in_maps.append({"blob256": blob256, "blob128": blob128})
    return in_maps


def run_spmd(x, w_qkv, w_out, b_out, **kw):
    nc = _get_nc()
    in_maps = make_in_maps(x, w_qkv, w_out, b_out)
    return run_bass_kernel_spmd(nc, in_maps, core_ids=list(range(8)), **kw)


def assemble(results):
    out = np.empty((4, C, NJ), np.float32)
    for core in range(8):
        b, qh = divmod(core, 2)
        out[b, :, qh * NI:(qh + 1) * NI] = results[core]["out_t"].T
    return out.reshape(4, C, 64, 64)


def kernel(x, w_qkv, w_out, b_out):
    res = run_spmd(x, w_qkv, w_out, b_out)
    return assemble(res.results)



# revision 6
# speedup vs baseline: 1.5478x; 1.5478x over previous
"""Trainium2 Bass kernel for nn_Attention (dense transformer spatial attention).

Reference computation (per batch b of 4):
  X = x[b] reshaped [256, 4096]                      (4096 = 64*64 pixels)
  QKV = w_qkv @ X -> [384, 4096]; q,k,v = split(QKV) each [128, 4096]
  per head h (4 heads x 32 dims): sim = (q_h*scale)^T k_h   [4096, 4096]
  attn = softmax(sim, axis=-1); out_h = attn @ v_h^T        [4096, 32]
  H = concat_heads -> [128, 4096]; out = w_out @ H + b_out  [256, 4096]

Sharding: 8 cores = (batch b in 0..3) x (query half qh in 0..1).
Each core gets full X_b (for K/V) plus its query-half slice, computes
attention output for its 2048 queries over all 4096 keys, and the final
projection. Host gather is concatenation + transpose.

Device algorithm (per core). The kernel streams 33.5M softmax exps per
core; throughput comes from splitting the exp work across THREE engines
and keeping the PE stream cost minimal:
  - sim is computed TRANSPOSED: simT[j, i] = sum_d k[d,j] q[d,i], one
    [128, 512] psum tile per head (4 tags), row-packed K=32 matmuls
    (tile_position=(32h, 0)), f32r operands (1 cyc/row at N>=256).
    The 4 single-buffered tags ping-pong so the engines never wait on
    a psum refill.
  - exp: heads 0/1 exact on ScalarE (Exp activation, scale folded,
    bf16 out). Heads 2/3 via the Schraudolph bit-trick on DVE / Pool:
    bf16 exp(x*scale) ~= bitcast(int16(x*(128*log2e*scale) + 127*128)),
    one tensor_scalar per tile writing through an int16 view. Max elem
    error ~6.5% one-sided; softmax num/denom cancellation + diffuse
    attention bring the end-to-end error to ~7e-3 of max|out| (the
    correctness gate is 2e-2). No max-subtraction (|scale*sim| < ~8).
  - AV is computed in the [i, hd] layout: av[i-tile, (h,d)] +=
    ex_h[j, i-tile]^T @ vTaug[j, d], lhsT = exp tile slice (stationary)
    so the moving operand is vT with N=33 -> 33 PE cycles per matmul
    instead of 512. vT is augmented with a ones column: col 32 of each
    head block accumulates the softmax denominator for free.
  - psum budget (8 banks): 4 sim tags + 2 av banks ([128,2,4,33] f32
    in a full-bank tile; 16 independent 132B accumulation groups per
    chunk share the banks -- only the first matmul of a bank uses
    start=True, everything else relies on the per-byte pending-zero
    overwrite semantics, group check skipped) + 1 bank shared by the
    transpose output (bf16, first half) and the projection psum
    (second half) + 1 bank for phase-1 q/k/vT projections.
  - epilogue per 512-query chunk: reciprocal of the 16 denominators
    (DVE), normalize av -> S[i, hd] bf16 (16 tensor_scalars, DVE+Pool),
    PE transpose via identity matmul -> T[hd, i] bf16 psum, copy to H
    sbuf, projection matmul lhsT=H_tile (K=128, all 4 heads at once,
    N=256 bf16), bias add (DVE/Pool), DMA out [i, o]; the host
    transposes. Epilogue slices are emitted interleaved with the next
    chunk's first steps so the PE instruction stream never stalls on
    the reciprocal chain.
"""

import numpy as np

import concourse.bacc as bacc
import concourse.bass as bass
import concourse.mybir as mybir
import concourse.tile as tile
from concourse.bass_utils import run_bass_kernel_spmd

F32 = mybir.dt.float32
F32R = mybir.dt.float32r
BF16 = mybir.dt.bfloat16
I16 = mybir.dt.int16

HEADS = 4
DH = 32                      # dim per head
C = 256                      # input channels
NJ = 4096                    # keys per batch (64*64)
NI = 2048                    # queries per core (half of 4096)
JT = 128                     # j tile (partition dim of simT)
NJT = NJ // JT               # 32 j tiles
NT = 512                     # i columns per step / chunk width
NCHUNK = NI // NT            # 4
SCALE = float(DH) ** -0.5
BW = 3 * 128 + NI + NJ       # blob256 width
# Schraudolph bf16-exp constants: bitcast(int16(x*EA + EB)) ~ exp(x*SCALE)
EA = float(SCALE * 128.0 / np.log(2.0))
EB = float(127 * 128)
AluOp = mybir.AluOpType


def build_kernel():
    nc = bacc.Bacc("TRN2", debug=False, num_devices=8)

    # blob256 columns: [wqkvT (384) | xq (2048) | x (4096)]
    # blob128 columns: [woutT bf16 (128 f32 words) | bias f32 (256) |
    #                   identity bf16 (64 f32 words)]
    blob256_d = nc.dram_tensor("blob256", [C, BW], F32R, kind="ExternalInput").ap()
    blob128_d = nc.dram_tensor("blob128", [128, 448], F32, kind="ExternalInput").ap()
    out_d = nc.dram_tensor("out_t", [NI, C], F32, kind="ExternalOutput").ap()

    with tile.TileContext(nc) as tc:
        with (
            tc.tile_pool(name="singles", bufs=1) as singles,
            tc.tile_pool(name="expp", bufs=4) as expp,
            tc.tile_pool(name="workp", bufs=2) as workp,
            tc.tile_pool(name="outp", bufs=4) as outp,
            tc.tile_pool(name="psim", bufs=1, space="PSUM") as psim,
            tc.tile_pool(name="pav", bufs=2, space="PSUM") as pav,
            tc.tile_pool(name="ptp", bufs=1, space="PSUM") as ptp,
            tc.tile_pool(name="pp1", bufs=1, space="PSUM") as pp1,
        ):
            # ---- resident SBUF tensors ----
            blob_sb = singles.tile([128, 2, BW], F32R)    # w|xq|x, 2 c-tiles
            w_sb = blob_sb[:, :, 0:3 * 128]
            xq_sb = blob_sb[:, :, 3 * 128:3 * 128 + NI]
            x_sb = blob_sb[:, :, 3 * 128 + NI:BW]
            b128_sb = singles.tile([128, 448], F32)
            woutT_bf = b128_sb[:, 0:128].bitcast(BF16)    # [128 hd, 256 o]
            bias_sb = b128_sb[:, 128:384]                 # [128, 256] replicated
            ident_bf = b128_sb[:, 384:448].bitcast(BF16)  # [128, 128] identity
            q_sb = singles.tile([128, NI], F32R)          # rows = 4h x 32d
            k_sb = singles.tile([128, NJ], F32R)
            vT_sb = singles.tile([128, NJT, HEADS, 33], BF16)  # [j, jt, h, d|1]
            h_sb = singles.tile([128, NI], BF16)          # [hd, i] normalized

            # single SWDGE queue; pieces ordered so q (w+xq0) and the first
            # k groups (x pieces) become available as early as possible.
            # each 512-col piece is one DMA per c-tile so every consumer
            # depends on few writers.
            PW = 512
            nc.sync.dma_start(out=blob_sb[:, 0, 0:384], in_=blob256_d[0:128, 0:384])
            nc.sync.dma_start(out=blob_sb[:, 1, 0:384], in_=blob256_d[128:256, 0:384])
            order = []
            for p in range(4):
                order.append(384 + p * PW)           # xq piece p
                order.append(384 + NI + p * PW)      # x piece p
            for p in range(4, 8):
                order.append(384 + NI + p * PW)      # x piece p
            for lo in order:
                for ct in range(2):
                    nc.sync.dma_start(
                        out=blob_sb[:, ct, lo:lo + PW],
                        in_=blob256_d[ct * 128:(ct + 1) * 128, lo:lo + PW])
            nc.sync.dma_start(out=b128_sb, in_=blob128_d)

            # ones column of vT (col 32 of each (jt, h) block)
            nc.gpsimd.memset(vT_sb[:, :, :, 32], 1.0)

            # warm the ScalarE exp table during the DMA wait
            warm = singles.tile([1, 1], F32)
            nc.gpsimd.memset(warm, 0.0)
            nc.scalar.activation(warm, warm, mybir.ActivationFunctionType.Exp)

            # ---- phase 1: q/k/vT projections (psum: pp1 + ptp banks) ----
            def emit_q_group(g, tag, eng):
                pool = pp1 if tag == "p1" else ptp
                ps = pool.tile([128, NT], F32, tag=tag, name="ph1q")
                for ct in range(2):
                    nc.tensor.matmul(
                        ps, lhsT=w_sb[:, ct, 0:128],
                        rhs=xq_sb[:, ct, g * NT:(g + 1) * NT],
                        start=(ct == 0), stop=(ct == 1))
                eng.tensor_copy(q_sb[:, g * NT:(g + 1) * NT], ps)

            def emit_k_group(g, tag, eng):
                pool = pp1 if tag == "p1" else ptp
                ps = pool.tile([128, NT], F32, tag=tag, name="ph1k")
                for ct in range(2):
                    nc.tensor.matmul(
                        ps, lhsT=w_sb[:, ct, 128:256],
                        rhs=x_sb[:, ct, g * NT:(g + 1) * NT],
                        start=(ct == 0), stop=(ct == 1))
                eng.tensor_copy(k_sb[:, g * NT:(g + 1) * NT], ps)

            def emit_vt_pair(jt, tag, eng):
                # two j-tiles at once: [128, 256] psum
                pool = pp1 if tag == "p1" else ptp
                ps = pool.tile([128, NT], F32, tag=tag, name="ph1v")
                for half in range(2):
                    o = half * 128
                    nc.tensor.matmul(
                        ps[:, o:o + 128],
                        lhsT=x_sb[:, 0, (jt + half) * JT:(jt + half + 1) * JT],
                        rhs=w_sb[:, 0, 256:384], start=True, stop=False)
                    nc.tensor.matmul(
                        ps[:, o:o + 128],
                        lhsT=x_sb[:, 1, (jt + half) * JT:(jt + half + 1) * JT],
                        rhs=w_sb[:, 1, 256:384], start=False, stop=True)
                eng.tensor_copy(
                    vT_sb[:, jt:jt + 2, :, 0:DH],
                    ps[:, 0:256].rearrange("p (a h d) -> p a h d", a=2, h=4))

            # serial phase 1, ordered to match DMA arrival; alternate the
            # two psum banks and the two copy engines for overlap
            ph1 = [("q", 0)]
            for g in range(8):
                ph1.append(("k", g))
                ph1.append(("vt", 4 * g))
                ph1.append(("vt", 4 * g + 2))
                if g in (1, 3, 5):
                    ph1.append(("q", (g + 1) // 2))
            for i, (kind, idx) in enumerate(ph1):
                tag = "p1" if i % 2 == 0 else "tp"
                eng = nc.vector
                if kind == "q":
                    emit_q_group(idx, tag, eng)
                elif kind == "k":
                    emit_k_group(idx, tag, eng)
                else:
                    emit_vt_pair(idx, tag, eng)

            # ---- main loop ----
            pending_av = None
            pending_epi = []

            def emit_av(ex_tiles, av_views, jt):
                first = (jt == 0)
                last = (jt == NJT - 1)
                for it in range(4):
                    av = av_views[it // 2]
                    for h in range(HEADS):
                        nc.tensor.matmul(
                            av[:, it % 2, h, :],
                            lhsT=ex_tiles[h][:, it * 128:(it + 1) * 128],
                            rhs=vT_sb[:, jt, h, :],
                            start=(first and it % 2 == 0 and h == 0),
                            stop=(last and it % 2 == 1 and h == 3),
                            skip_group_check=True)

            def make_epilogue(chunk, av_views):
                co = chunk * NT
                den = workp.tile([128, 16], F32, tag="den")
                rc = workp.tile([128, 16], F32, tag="rc")
                s_t = workp.tile([128, 4, 128], BF16, tag="s")
                tp = ptp.tile([128, 512], F32, tag="tp")
                t_bf = tp[:, 0:256].bitcast(BF16)        # [128, 512] T
                pj = tp[:, 256:512]                      # [128, 256] proj psum

                def p_recip():
                    for i, av in enumerate(av_views):
                        nc.vector.tensor_copy(
                            den[:, i * 8:(i + 1) * 8],
                            av[:, :, :, 32].rearrange("p a b -> p (a b)"))
                    nc.vector.reciprocal(out=rc, in_=den)

                def p_norm(half):
                    for it in range(2 * half, 2 * half + 2):
                        av = av_views[it // 2]
                        for h in range(HEADS):
                            if half == 0:
                                nc.vector.tensor_scalar(
                                    out=s_t[:, it, h * DH:(h + 1) * DH],
                                    in0=av[:, it % 2, h, 0:DH],
                                    scalar1=rc[:, it * 4 + h:it * 4 + h + 1],
                                    scalar2=None, op0=AluOp.mult)
                            else:
                                nc.scalar.activation(
                                    s_t[:, it, h * DH:(h + 1) * DH],
                                    av[:, it % 2, h, 0:DH],
                                    mybir.ActivationFunctionType.Copy,
                                    scale=rc[:, it * 4 + h:it * 4 + h + 1])

                def p_trans():
                    for it in range(4):
                        nc.tensor.transpose(
                            t_bf[:, it * 128:(it + 1) * 128],
                            s_t[:, it, :], ident_bf)
                    nc.vector.tensor_copy(h_sb[:, co:co + NT], t_bf)

                def p_proj():
                    for it in range(4):
                        io = co + it * 128
                        nc.tensor.matmul(pj, lhsT=h_sb[:, io:io + 128],
                                         rhs=woutT_bf, start=True, stop=True)
                        ot = outp.tile([128, C], F32, tag="out")
                        nc.vector.tensor_tensor(out=ot, in0=pj, in1=bias_sb,
                                                op=AluOp.add)
                        nc.sync.dma_start(out=out_d[io:io + 128, :], in_=ot)

                return [p_recip, lambda: p_norm(0), lambda: p_norm(1),
                        p_trans, p_proj]

            for chunk in range(NCHUNK):
                co = chunk * NT
                av_views = []
                for _ in range(2):
                    t = pav.tile([128, 512], F32, tag="av", name="avt")
                    av_views.append(
                        t[:, 0:264].rearrange("p (a h d) -> p a h d", a=2, h=4))
                for jt in range(NJT):
                    sims = []
                    for h in range(HEADS):
                        s = psim.tile([128, NT], F32, tag=f"s{h}")
                        nc.tensor.matmul(
                            s, lhsT=k_sb[h * DH:(h + 1) * DH,
                                         jt * JT:(jt + 1) * JT],
                            rhs=q_sb[h * DH:(h + 1) * DH, co:co + NT],
                            start=True, stop=True,
                            tile_position=(h * DH, 0))
                        sims.append(s)
                    exs = [expp.tile([128, NT], BF16, tag=f"e{h}", name=f"ex{h}")
                           for h in range(HEADS)]
                    nc.scalar.activation(exs[0], sims[0],
                                         mybir.ActivationFunctionType.Exp,
                                         scale=SCALE)
                    nc.scalar.activation(exs[1], sims[1],
                                         mybir.ActivationFunctionType.Exp,
                                         scale=SCALE)
                    nc.vector.tensor_scalar(
                        out=exs[2].bitcast(I16), in0=sims[2],
                        scalar1=EA, scalar2=EB, op0=AluOp.mult, op1=AluOp.add)
                    nc.vector.tensor_scalar(
                        out=exs[3].bitcast(I16), in0=sims[3],
                        scalar1=EA, scalar2=EB, op0=AluOp.mult, op1=AluOp.add)

                    # previous step's AV after this step's sims (PE order)
                    if pending_av is not None:
                        emit_av(*pending_av)
                    pending_av = (exs, av_views, jt)

                    if pending_epi:
                        pending_epi.pop(0)()

                emit_av(*pending_av)
                pending_av = None
                pending_epi = make_epilogue(chunk, av_views)
            while pending_epi:
                pending_epi.pop(0)()

    nc.compile()
    return nc


_NC = None


def _get_nc():
    global _NC
    if _NC is None:
        _NC = build_kernel()
    return _NC


def make_in_maps(x, w_qkv, w_out, b_out):
    import ml_dtypes
    x = np.ascontiguousarray(np.asarray(x, dtype=np.float32))
    w_qkv = np.asarray(w_qkv, dtype=np.float32)
    w_out = np.asarray(w_out, dtype=np.float32)
    b_out = np.asarray(b_out, dtype=np.float32)

    wqkvT = w_qkv.T                                       # [256, 384]
    woutT = w_out.T                                       # [128 hd, 256 o]

    def pack_bf16(a):
        bf = a.astype(ml_dtypes.bfloat16).view(np.uint16)
        lo = bf[:, 0::2].astype(np.uint32)
        hi = bf[:, 1::2].astype(np.uint32)
        return (lo | (hi << 16)).view(np.float32)

    blob128 = np.ascontiguousarray(np.concatenate([
        pack_bf16(woutT),                                  # 128 cols
        np.broadcast_to(b_out[None, :], (128, C)),         # 256 cols
        pack_bf16(np.eye(128, dtype=np.float32)),          # 64 cols
    ], axis=1, dtype=np.float32))

    in_maps = []
    for core in range(8):
        b, qh = divmod(core, 2)
        xb = x[b].reshape(C, NJ)
        xqb = xb[:, qh * NI:(qh + 1) * NI]
        blob256 = np.ascontiguousarray(
            np.concatenate([wqkvT, xqb, xb], axis=1))
        in_maps.append({"blob256": blob256, "blob128": blob128})
    return in_maps


def run_spmd(x, w_qkv, w_out, b_out, **kw):
    nc = _get_nc()
    in_maps = make_in_maps(x, w_qkv, w_out, b_out)
    return run_bass_kernel_spmd(nc, in_maps, core_ids=list(range(8)), **kw)


def assemble(results):
    out = np.empty((4, C, NJ), np.float32)
    for core in range(8):
        b, qh = divmod(core, 2)
        out[b, :, qh * NI:(qh + 1) * NI] = results[core]["out_t"].T
    return out.reshape(4, C, 64, 64)


def kernel(x, w_qkv, w_out, b_out):
    res = run_spmd(x, w_qkv, w_out, b_out)
    return assemble(res.results)


# revision 8
# speedup vs baseline: 1.5783x; 1.0197x over previous
"""Trainium2 Bass kernel for nn_Attention (dense transformer spatial attention).

Reference computation (per batch b of 4):
  X = x[b] reshaped [256, 4096]                      (4096 = 64*64 pixels)
  QKV = w_qkv @ X -> [384, 4096]; q,k,v = split(QKV) each [128, 4096]
  per head h (4 heads x 32 dims): sim = (q_h*scale)^T k_h   [4096, 4096]
  attn = softmax(sim, axis=-1); out_h = attn @ v_h^T        [4096, 32]
  H = concat_heads -> [128, 4096]; out = w_out @ H + b_out  [256, 4096]

Sharding: 8 cores = (batch b in 0..3) x (query half qh in 0..1).
Each core gets full X_b (for K/V) plus its query-half slice, computes
attention output for its 2048 queries over all 4096 keys, and the final
projection. Host gather is concatenation + transpose.

Device algorithm (per core). The kernel streams 33.5M softmax exps per
core; throughput comes from splitting the exp work across THREE engines
and keeping the PE stream cost minimal:
  - sim is computed TRANSPOSED: simT[j, i] = sum_d k[d,j] q[d,i], one
    [128, 512] psum tile per head (4 tags), row-packed K=32 matmuls
    (tile_position=(32h, 0)), f32r operands (1 cyc/row at N>=256).
    The 4 single-buffered tags ping-pong so the engines never wait on
    a psum refill.
  - exp: heads 0/1 exact on ScalarE (Exp activation, scale folded,
    bf16 out). Heads 2/3 via the Schraudolph bit-trick on DVE / Pool:
    bf16 exp(x*scale) ~= bitcast(int16(x*(128*log2e*scale) + 127*128)),
    one tensor_scalar per tile writing through an int16 view. Max elem
    error ~6.5% one-sided; softmax num/denom cancellation + diffuse
    attention bring the end-to-end error to ~7e-3 of max|out| (the
    correctness gate is 2e-2). No max-subtraction (|scale*sim| < ~8).
  - AV is computed in the [i, hd] layout: av[i-tile, (h,d)] +=
    ex_h[j, i-tile]^T @ vTaug[j, d], lhsT = exp tile slice (stationary)
    so the moving operand is vT with N=33 -> 33 PE cycles per matmul
    instead of 512. vT is augmented with a ones column: col 32 of each
    head block accumulates the softmax denominator for free.
  - psum budget (8 banks): 4 sim tags + 2 av banks ([128,2,4,33] f32
    in a full-bank tile; 16 independent 132B accumulation groups per
    chunk share the banks -- only the first matmul of a bank uses
    start=True, everything else relies on the per-byte pending-zero
    overwrite semantics, group check skipped) + 1 bank shared by the
    transpose output (bf16, first half) and the projection psum
    (second half) + 1 bank for phase-1 q/k/vT projections.
  - epilogue per 512-query chunk: reciprocal of the 16 denominators
    (DVE), normalize av -> S[i, hd] bf16 (16 tensor_scalars, DVE+Pool),
    PE transpose via identity matmul -> T[hd, i] bf16 psum, copy to H
    sbuf, projection matmul lhsT=H_tile (K=128, all 4 heads at once,
    N=256 bf16), bias add (DVE/Pool), DMA out [i, o]; the host
    transposes. Epilogue slices are emitted interleaved with the next
    chunk's first steps so the PE instruction stream never stalls on
    the reciprocal chain.
"""

import numpy as np

import concourse.bacc as bacc
import concourse.bass as bass
import concourse.mybir as mybir
import concourse.tile as tile
from concourse.bass_utils import run_bass_kernel_spmd

F32 = mybir.dt.float32
F32R = mybir.dt.float32r
BF16 = mybir.dt.bfloat16
I16 = mybir.dt.int16

HEADS = 4
DH = 32                      # dim per head
C = 256                      # input channels
NJ = 4096                    # keys per batch (64*64)
NI = 2048                    # queries per core (half of 4096)
JT = 128                     # j tile (partition dim of simT)
NJT = NJ // JT               # 32 j tiles
NT = 512                     # i columns per step / chunk width
NCHUNK = NI // NT            # 4
SCALE = float(DH) ** -0.5
BW = 3 * 128 + NI + NJ       # blob256 width
# Schraudolph bf16-exp constants: bitcast(int16(x*EA + EB)) ~ exp(x*SCALE)
EA = float(SCALE * 128.0 / np.log(2.0))
EB = float(127 * 128)
AluOp = mybir.AluOpType


def build_kernel():
    nc = bacc.Bacc("TRN2", debug=False, num_devices=8)

    # blob256 columns: [wqkvT (384) | xq (2048) | x (4096)]
    # blob128 columns: [woutT bf16 (128 f32 words) | bias f32 (256) |
    #                   identity bf16 (64 f32 words)]
    blob256_d = nc.dram_tensor("blob256", [C, BW], F32R, kind="ExternalInput").ap()
    blob128_d = nc.dram_tensor("blob128", [128, 448], F32, kind="ExternalInput").ap()
    out_d = nc.dram_tensor("out_t", [NI, C], F32, kind="ExternalOutput").ap()

    with tile.TileContext(nc) as tc:
        with (
            tc.tile_pool(name="singles", bufs=1) as singles,
            tc.tile_pool(name="expp", bufs=4) as expp,
            tc.tile_pool(name="workp", bufs=2) as workp,
            tc.tile_pool(name="outp", bufs=4) as outp,
            tc.tile_pool(name="psim", bufs=1, space="PSUM") as psim,
            tc.tile_pool(name="pav", bufs=2, space="PSUM") as pav,
            tc.tile_pool(name="ptp", bufs=1, space="PSUM") as ptp,
            tc.tile_pool(name="pp1", bufs=1, space="PSUM") as pp1,
        ):
            # ---- resident SBUF tensors ----
            blob_sb = singles.tile([128, 2, BW], F32R)    # w|xq|x, 2 c-tiles
            w_sb = blob_sb[:, :, 0:3 * 128]
            xq_sb = blob_sb[:, :, 3 * 128:3 * 128 + NI]
            x_sb = blob_sb[:, :, 3 * 128 + NI:BW]
            b128_sb = singles.tile([128, 448], F32)
            woutT_bf = b128_sb[:, 0:128].bitcast(BF16)    # [128 hd, 256 o]
            bias_sb = b128_sb[:, 128:384]                 # [128, 256] replicated
            ident_bf = b128_sb[:, 384:448].bitcast(BF16)  # [128, 128] identity
            q_sb = singles.tile([128, NI], F32R)          # rows = 4h x 32d
            k_sb = singles.tile([128, NJ], F32R)
            vT_sb = singles.tile([128, NJT, HEADS, 33], BF16)  # [j, jt, h, d|1]
            h_sb = singles.tile([128, NI], BF16)          # [hd, i] normalized

            # single SWDGE queue; pieces ordered so q (w+xq0) and the first
            # k groups (x pieces) become available as early as possible.
            # each 512-col piece is one DMA per c-tile so every consumer
            # depends on few writers.
            PW = 512
            nc.sync.dma_start(out=blob_sb[:, 0, 0:384], in_=blob256_d[0:128, 0:384])
            nc.sync.dma_start(out=blob_sb[:, 1, 0:384], in_=blob256_d[128:256, 0:384])
            order = []
            for p in range(4):
                order.append(384 + p * PW)           # xq piece p
                order.append(384 + NI + p * PW)      # x piece p
            for p in range(4, 8):
                order.append(384 + NI + p * PW)      # x piece p
            for lo in order:
                for ct in range(2):
                    nc.sync.dma_start(
                        out=blob_sb[:, ct, lo:lo + PW],
                        in_=blob256_d[ct * 128:(ct + 1) * 128, lo:lo + PW])
            nc.sync.dma_start(out=b128_sb, in_=blob128_d)

            # ones column of vT (col 32 of each (jt, h) block)
            nc.gpsimd.memset(vT_sb[:, :, :, 32], 1.0)

            # warm the ScalarE exp table during the DMA wait
            warm = singles.tile([1, 1], F32)
            nc.gpsimd.memset(warm, 0.0)
            nc.scalar.activation(warm, warm, mybir.ActivationFunctionType.Exp)

            # ---- phase 1: q/k/vT projections (psum: pp1 + ptp banks) ----
            def emit_q_group(g, tag):
                pool = pp1 if tag == "p1" else ptp
                ps = pool.tile([128, NT], F32, tag=tag, name="ph1q")
                for ct in range(2):
                    nc.tensor.matmul(
                        ps, lhsT=w_sb[:, ct, 0:128],
                        rhs=xq_sb[:, ct, g * NT:(g + 1) * NT],
                        start=(ct == 0), stop=(ct == 1))
                nc.scalar.activation(
                    q_sb[:, g * NT:(g + 1) * NT], ps,
                    mybir.ActivationFunctionType.Copy)

            def emit_k_group(g, tag):
                pool = pp1 if tag == "p1" else ptp
                ps = pool.tile([128, NT], F32, tag=tag, name="ph1k")
                for ct in range(2):
                    nc.tensor.matmul(
                        ps, lhsT=w_sb[:, ct, 128:256],
                        rhs=x_sb[:, ct, g * NT:(g + 1) * NT],
                        start=(ct == 0), stop=(ct == 1))
                nc.scalar.activation(
                    k_sb[:, g * NT:(g + 1) * NT], ps,
                    mybir.ActivationFunctionType.Copy)

            def emit_vt_pair(jt, tag):
                # two j-tiles at once: [128, 256] psum
                pool = pp1 if tag == "p1" else ptp
                ps = pool.tile([128, NT], F32, tag=tag, name="ph1v")
                for half in range(2):
                    o = half * 128
                    nc.tensor.matmul(
                        ps[:, o:o + 128],
                        lhsT=x_sb[:, 0, (jt + half) * JT:(jt + half + 1) * JT],
                        rhs=w_sb[:, 0, 256:384], start=True, stop=False)
                    nc.tensor.matmul(
                        ps[:, o:o + 128],
                        lhsT=x_sb[:, 1, (jt + half) * JT:(jt + half + 1) * JT],
                        rhs=w_sb[:, 1, 256:384], start=False, stop=True)
                nc.scalar.activation(
                    vT_sb[:, jt:jt + 2, :, 0:DH],
                    ps[:, 0:256].rearrange("p (a h d) -> p a h d", a=2, h=4),
                    mybir.ActivationFunctionType.Copy)

            # phase 1 is interleaved into chunk 0: prelude emits just what
            # the first steps need; the rest is scheduled per step to match
            # DMA arrival, alternating the two spare psum banks
            ph1_ctr = [0]

            def emit_phase1(kind, idx):
                tag = "p1" if ph1_ctr[0] % 2 == 0 else "tp"
                ph1_ctr[0] += 1
                if kind == "q":
                    emit_q_group(idx, tag)
                elif kind == "k":
                    emit_k_group(idx, tag)
                else:
                    emit_vt_pair(idx, tag)

            # work scheduled before chunk-0 step jt
            ph1_step = {}
            for jt in range(NJT):
                work = []
                if jt % 4 == 0 and jt // 4 + 1 < 8:
                    work.append(("k", jt // 4 + 1))
                if jt % 2 == 0 and jt + 2 < NJT:
                    work.append(("vt", jt + 2))
                if jt in (10, 14, 18):
                    work.append(("q", (jt - 6) // 4))
                ph1_step[jt] = work

            emit_phase1("q", 0)
            emit_phase1("k", 0)
            emit_phase1("vt", 0)

            # ---- main loop ----
            pending_av = None
            pending_epi = []

            def emit_av(ex_tiles, av_views, jt):
                first = (jt == 0)
                last = (jt == NJT - 1)
                for it in range(4):
                    av = av_views[it // 2]
                    for h in range(HEADS):
                        nc.tensor.matmul(
                            av[:, it % 2, h, :],
                            lhsT=ex_tiles[h][:, it * 128:(it + 1) * 128],
                            rhs=vT_sb[:, jt, h, :],
                            start=(first and it % 2 == 0 and h == 0),
                            stop=(last and it % 2 == 1 and h == 3),
                            skip_group_check=True)

            def make_epilogue(chunk, av_views):
                co = chunk * NT
                den = workp.tile([128, 16], F32, tag="den")
                rc = workp.tile([128, 16], F32, tag="rc")
                s_t = workp.tile([128, 4, 128], BF16, tag="s")
                tp = ptp.tile([128, 512], F32, tag="tp")
                t_bf = tp[:, 0:256].bitcast(BF16)        # [128, 512] T
                pj = tp[:, 256:512]                      # [128, 256] proj psum

                def p_recip():
                    for i, av in enumerate(av_views):
                        nc.vector.tensor_copy(
                            den[:, i * 8:(i + 1) * 8],
                            av[:, :, :, 32].rearrange("p a b -> p (a b)"))
                    nc.vector.reciprocal(out=rc, in_=den)

                def p_norm(half):
                    for it in range(2 * half, 2 * half + 2):
                        av = av_views[it // 2]
                        for h in range(HEADS):
                            if half == 0:
                                nc.vector.tensor_scalar(
                                    out=s_t[:, it, h * DH:(h + 1) * DH],
                                    in0=av[:, it % 2, h, 0:DH],
                                    scalar1=rc[:, it * 4 + h:it * 4 + h + 1],
                                    scalar2=None, op0=AluOp.mult)
                            else:
                                nc.scalar.activation(
                                    s_t[:, it, h * DH:(h + 1) * DH],
                                    av[:, it % 2, h, 0:DH],
                                    mybir.ActivationFunctionType.Copy,
                                    scale=rc[:, it * 4 + h:it * 4 + h + 1])

                def p_trans():
                    for it in range(4):
                        nc.tensor.transpose(
                            t_bf[:, it * 128:(it + 1) * 128],
                            s_t[:, it, :], ident_bf)
                    nc.vector.tensor_copy(h_sb[:, co:co + NT], t_bf)

                def p_proj():
                    for it in range(4):
                        io = co + it * 128
                        nc.tensor.matmul(pj, lhsT=h_sb[:, io:io + 128],
                                         rhs=woutT_bf, start=True, stop=True)
                        ot = outp.tile([128, C], F32, tag="out")
                        nc.vector.tensor_tensor(out=ot, in0=pj, in1=bias_sb,
                                                op=AluOp.add)
                        nc.sync.dma_start(out=out_d[io:io + 128, :], in_=ot)

                return [p_recip, lambda: p_norm(0), lambda: p_norm(1),
                        p_trans, p_proj]

            for chunk in range(NCHUNK):
                co = chunk * NT
                av_views = []
                for _ in range(2):
                    t = pav.tile([128, 512], F32, tag="av", name="avt")
                    av_views.append(
                        t[:, 0:264].rearrange("p (a h d) -> p a h d", a=2, h=4))
                for jt in range(NJT):
                    sims = []
                    for h in range(HEADS):
                        s = psim.tile([128, NT], F32, tag=f"s{h}")
                        nc.tensor.matmul(
                            s, lhsT=k_sb[h * DH:(h + 1) * DH,
                                         jt * JT:(jt + 1) * JT],
                            rhs=q_sb[h * DH:(h + 1) * DH, co:co + NT],
                            start=True, stop=True,
                            tile_position=(h * DH, 0))
                        sims.append(s)
                    exs = [expp.tile([128, NT], BF16, tag=f"e{h}", name=f"ex{h}")
                           for h in range(HEADS)]
                    nc.scalar.activation(exs[0], sims[0],
                                         mybir.ActivationFunctionType.Exp,
                                         scale=SCALE)
                    nc.scalar.activation(exs[1], sims[1],
                                         mybir.ActivationFunctionType.Exp,
                                         scale=SCALE)
                    nc.vector.tensor_scalar(
                        out=exs[2].bitcast(I16), in0=sims[2],
                        scalar1=EA, scalar2=EB, op0=AluOp.mult, op1=AluOp.add)
                    nc.vector.tensor_scalar(
                        out=exs[3].bitcast(I16), in0=sims[3],
                        scalar1=EA, scalar2=EB, op0=AluOp.mult, op1=AluOp.add)

                    # previous step's AV after this step's sims (PE order)
                    if pending_av is not None:
                        emit_av(*pending_av)
                    pending_av = (exs, av_views, jt)

                    if pending_epi:
                        pending_epi.pop(0)()
                    if chunk == 0:
                        for kind, idx in ph1_step.get(jt, []):
                            emit_phase1(kind, idx)

                emit_av(*pending_av)
                pending_av = None
                pending_epi = make_epilogue(chunk, av_views)
            while pending_epi:
                pending_epi.pop(0)()

    nc.compile()
    return nc


_NC = None


def _get_nc():
    global _NC
    if _NC is None:
        _NC = build_kernel()
    return _NC


def make_in_maps(x, w_qkv, w_out, b_out):
    import ml_dtypes
    x = np.ascontiguousarray(np.asarray(x, dtype=np.float32))
    w_qkv = np.asarray(w_qkv, dtype=np.float32)
    w_out = np.asarray(w_out, dtype=np.float32)
    b_out = np.asarray(b_out, dtype=np.float32)

    wqkvT = w_qkv.T                                       # [256, 384]
    woutT = w_out.T                                       # [128 hd, 256 o]

    def pack_bf16(a):
        bf = a.astype(ml_dtypes.bfloat16).view(np.uint16)
        lo = bf[:, 0::2].astype(np.uint32)
        hi = bf[:, 1::2].astype(np.uint32)
        return (lo | (hi << 16)).view(np.float32)

    blob128 = np.ascontiguousarray(np.concatenate([
        pack_bf16(woutT),                                  # 128 cols
        np.broadcast_to(b_out[None, :], (128, C)),         # 256 cols
        pack_bf16(np.eye(128, dtype=np.float32)),          # 64 cols
    ], axis=1, dtype=np.float32))

    in_maps = []
    for core in range(8):
        b, qh = divmod(core, 2)
        xb = x[b].reshape(C, NJ)
        xqb = xb[:, qh * NI:(qh + 1) * NI]
        blob256 = np.ascontiguousarray(
            np.concatenate([wqkvT, xqb, xb], axis=1))
        in_maps.append({"blob256": blob256, "blob128": blob128})
    return in_maps


def run_spmd(x, w_qkv, w_out, b_out, **kw):
    nc = _get_nc()
    in_maps = make_in_maps(x, w_qkv, w_out, b_out)
    return run_bass_kernel_spmd(nc, in_maps, core_ids=list(range(8)), **kw)


def assemble(results):
    out = np.empty((4, C, NJ), np.float32)
    for core in range(8):
        b, qh = divmod(core, 2)
        out[b, :, qh * NI:(qh + 1) * NI] = results[core]["out_t"].T
    return out.reshape(4, C, 64, 64)


def kernel(x, w_qkv, w_out, b_out):
    res = run_spmd(x, w_qkv, w_out, b_out)
    return assemble(res.results)


# revision 9
# speedup vs baseline: 1.5897x; 1.0073x over previous
"""Trainium2 Bass kernel for nn_Attention (dense transformer spatial attention).

Reference computation (per batch b of 4):
  X = x[b] reshaped [256, 4096]                      (4096 = 64*64 pixels)
  QKV = w_qkv @ X -> [384, 4096]; q,k,v = split(QKV) each [128, 4096]
  per head h (4 heads x 32 dims): sim = (q_h*scale)^T k_h   [4096, 4096]
  attn = softmax(sim, axis=-1); out_h = attn @ v_h^T        [4096, 32]
  H = concat_heads -> [128, 4096]; out = w_out @ H + b_out  [256, 4096]

Sharding: 8 cores = (batch b in 0..3) x (query half qh in 0..1).
Each core gets full X_b (for K/V) plus its query-half slice, computes
attention output for its 2048 queries over all 4096 keys, and the final
projection. Host gather is concatenation + transpose.

Device algorithm (per core). The kernel streams 33.5M softmax exps per
core; throughput comes from splitting the exp work across THREE engines
and keeping the PE stream cost minimal:
  - sim is computed TRANSPOSED: simT[j, i] = sum_d k[d,j] q[d,i], one
    [128, 512] psum tile per head (4 tags), row-packed K=32 matmuls
    (tile_position=(32h, 0)), f32r operands (1 cyc/row at N>=256).
    The 4 single-buffered tags ping-pong so the engines never wait on
    a psum refill.
  - exp: heads 0/1 exact on ScalarE (Exp activation, scale folded,
    bf16 out). Heads 2/3 via the Schraudolph bit-trick on DVE / Pool:
    bf16 exp(x*scale) ~= bitcast(int16(x*(128*log2e*scale) + 127*128)),
    one tensor_scalar per tile writing through an int16 view. Max elem
    error ~6.5% one-sided; softmax num/denom cancellation + diffuse
    attention bring the end-to-end error to ~7e-3 of max|out| (the
    correctness gate is 2e-2). No max-subtraction (|scale*sim| < ~8).
  - AV is computed in the [i, hd] layout: av[i-tile, (h,d)] +=
    ex_h[j, i-tile]^T @ vTaug[j, d], lhsT = exp tile slice (stationary)
    so the moving operand is vT with N=33 -> 33 PE cycles per matmul
    instead of 512. vT is augmented with a ones column: col 32 of each
    head block accumulates the softmax denominator for free.
  - psum budget (8 banks): 4 sim tags + 2 av banks ([128,2,4,33] f32
    in a full-bank tile; 16 independent 132B accumulation groups per
    chunk share the banks -- only the first matmul of a bank uses
    start=True, everything else relies on the per-byte pending-zero
    overwrite semantics, group check skipped) + 1 bank shared by the
    transpose output (bf16, first half) and the projection psum
    (second half) + 1 bank for phase-1 q/k/vT projections.
  - epilogue per 512-query chunk: reciprocal of the 16 denominators
    (DVE), normalize av -> S[i, hd] bf16 (16 tensor_scalars, DVE+Pool),
    PE transpose via identity matmul -> T[hd, i] bf16 psum, copy to H
    sbuf, projection matmul lhsT=H_tile (K=128, all 4 heads at once,
    N=256 bf16), bias add (DVE/Pool), DMA out [i, o]; the host
    transposes. Epilogue slices are emitted interleaved with the next
    chunk's first steps so the PE instruction stream never stalls on
    the reciprocal chain.
"""

import numpy as np

import concourse.bacc as bacc
import concourse.bass as bass
import concourse.mybir as mybir
import concourse.tile as tile
from concourse.bass_utils import run_bass_kernel_spmd

F32 = mybir.dt.float32
F32R = mybir.dt.float32r
BF16 = mybir.dt.bfloat16
I16 = mybir.dt.int16

HEADS = 4
DH = 32                      # dim per head
C = 256                      # input channels
NJ = 4096                    # keys per batch (64*64)
NI = 2048                    # queries per core (half of 4096)
JT = 128                     # j tile (partition dim of simT)
NJT = NJ // JT               # 32 j tiles
NT = 512                     # i columns per step / chunk width
NCHUNK = NI // NT            # 4
SCALE = float(DH) ** -0.5
BW = 3 * 128 + NJ            # blob256 width (w | x; q reads the qh half of x)
# Schraudolph bf16-exp constants: bitcast(int16(x*EA + EB)) ~ exp(x*SCALE)
EA = float(SCALE * 128.0 / np.log(2.0))
EB = float(127 * 128)
AluOp = mybir.AluOpType


def build_kernel():
    nc = bacc.Bacc("TRN2", debug=False, num_devices=8)

    # blob256 columns: [wqkvT (384) | x (4096)]; the query slice is read
    # in place from x (cores differ only in the qh column offset)
    # blob128 columns: [woutT bf16 (128 f32 words) | bias-row bf16 (128) |
    #                   identity bf16 (64 f32 words) | ones-row bf16 (64)]
    # bias-row/ones-row live on partition 0 only (K=1 matmul operands)
    blob256_d = nc.dram_tensor("blob256", [C, BW], F32R, kind="ExternalInput").ap()
    blob128_d = nc.dram_tensor("blob128", [128, 384], F32, kind="ExternalInput").ap()
    out_d = nc.dram_tensor("out_t", [NI, C], F32, kind="ExternalOutput").ap()

    with tile.TileContext(nc) as tc:
        with (
            tc.tile_pool(name="singles", bufs=1) as singles,
            tc.tile_pool(name="expp", bufs=4) as expp,
            tc.tile_pool(name="workp", bufs=2) as workp,
            tc.tile_pool(name="outp", bufs=4) as outp,
            tc.tile_pool(name="psim", bufs=1, space="PSUM") as psim,
            tc.tile_pool(name="pav", bufs=2, space="PSUM") as pav,
            tc.tile_pool(name="ptp", bufs=1, space="PSUM") as ptp,
            tc.tile_pool(name="pp1", bufs=1, space="PSUM") as pp1,
        ):
            # ---- resident SBUF tensors ----
            blob_sb = singles.tile([128, 2, BW], F32R)    # w|x, 2 c-tiles
            w_sb = blob_sb[:, :, 0:3 * 128]
            x_sb = blob_sb[:, :, 3 * 128:BW]
            b128_sb = singles.tile([128, 384], F32)
            woutT_bf = b128_sb[:, 0:128].bitcast(BF16)    # [128 hd, 256 o]
            bias_row = b128_sb[0:1, 128:256].bitcast(BF16)   # [1, 256]
            ident_bf = b128_sb[:, 256:320].bitcast(BF16)  # [128, 128] identity
            ones_row = b128_sb[0:1, 320:384].bitcast(BF16)   # [1, 128]
            q_sb = singles.tile([128, NI], F32R)          # rows = 4h x 32d
            k_sb = singles.tile([128, NJ], F32R)
            vT_sb = singles.tile([128, NJT, HEADS, 33], BF16)  # [j, jt, h, d|1]
            h_sb = singles.tile([128, NI], BF16)          # [hd, i] normalized

            # single SWDGE queue; pieces ordered so q (w+xq0) and the first
            # k groups (x pieces) become available as early as possible.
            # each 512-col piece is one DMA per c-tile so every consumer
            # depends on few writers.
            PW = 512
            nc.sync.dma_start(out=blob_sb[:, 0, 0:384], in_=blob256_d[0:128, 0:384])
            nc.sync.dma_start(out=blob_sb[:, 1, 0:384], in_=blob256_d[128:256, 0:384])
            order = []
            for p in range(4):
                order.append(384 + p * PW)           # query-half piece p
                order.append(384 + NI + p * PW)      # other-half piece p
            for lo in order:
                for ct in range(2):
                    nc.sync.dma_start(
                        out=blob_sb[:, ct, lo:lo + PW],
                        in_=blob256_d[ct * 128:(ct + 1) * 128, lo:lo + PW])
            nc.sync.dma_start(out=b128_sb, in_=blob128_d)

            # ones column of vT (col 32 of each (jt, h) block)
            nc.gpsimd.memset(vT_sb[:, :, :, 32], 1.0)

            # warm the ScalarE exp table during the DMA wait
            warm = singles.tile([1, 1], F32)
            nc.gpsimd.memset(warm, 0.0)
            nc.scalar.activation(warm, warm, mybir.ActivationFunctionType.Exp)

            # ---- phase 1: q/k/vT projections (psum: pp1 + ptp banks) ----
            def emit_q_group(g, tag, on_act):
                pool = pp1 if tag == "p1" else ptp
                ps = pool.tile([128, NT], F32, tag=tag, name="ph1q")
                for ct in range(2):
                    nc.tensor.matmul(
                        ps, lhsT=w_sb[:, ct, 0:128],
                        rhs=x_sb[:, ct, g * NT:(g + 1) * NT],
                        start=(ct == 0), stop=(ct == 1))
                if on_act:
                    nc.scalar.activation(
                        q_sb[:, g * NT:(g + 1) * NT], ps,
                        mybir.ActivationFunctionType.Copy)
                else:
                    nc.vector.tensor_copy(q_sb[:, g * NT:(g + 1) * NT], ps)

            def emit_k_group(g, tag, on_act):
                pool = pp1 if tag == "p1" else ptp
                ps = pool.tile([128, NT], F32, tag=tag, name="ph1k")
                for ct in range(2):
                    nc.tensor.matmul(
                        ps, lhsT=w_sb[:, ct, 128:256],
                        rhs=x_sb[:, ct, g * NT:(g + 1) * NT],
                        start=(ct == 0), stop=(ct == 1))
                if on_act:
                    nc.scalar.activation(
                        k_sb[:, g * NT:(g + 1) * NT], ps,
                        mybir.ActivationFunctionType.Copy)
                else:
                    nc.vector.tensor_copy(k_sb[:, g * NT:(g + 1) * NT], ps)

            def emit_vt_pair(jt, tag, on_act):
                # two j-tiles at once: [128, 256] psum
                pool = pp1 if tag == "p1" else ptp
                ps = pool.tile([128, NT], F32, tag=tag, name="ph1v")
                for half in range(2):
                    o = half * 128
                    nc.tensor.matmul(
                        ps[:, o:o + 128],
                        lhsT=x_sb[:, 0, (jt + half) * JT:(jt + half + 1) * JT],
                        rhs=w_sb[:, 0, 256:384], start=True, stop=False)
                    nc.tensor.matmul(
                        ps[:, o:o + 128],
                        lhsT=x_sb[:, 1, (jt + half) * JT:(jt + half + 1) * JT],
                        rhs=w_sb[:, 1, 256:384], start=False, stop=True)
                if on_act:
                    nc.scalar.activation(
                        vT_sb[:, jt:jt + 2, :, 0:DH],
                        ps[:, 0:256].rearrange("p (a h d) -> p a h d", a=2, h=4),
                        mybir.ActivationFunctionType.Copy)
                else:
                    nc.vector.tensor_copy(
                        vT_sb[:, jt:jt + 2, :, 0:DH],
                        ps[:, 0:256].rearrange("p (a h d) -> p a h d", a=2, h=4))

            # phase 1 is interleaved into chunk 0: prelude emits just what
            # the first steps need; the rest is scheduled per step to match
            # DMA arrival, alternating the two spare psum banks
            ph1_ctr = [0]

            def emit_phase1(kind, idx):
                tag = "p1" if ph1_ctr[0] % 2 == 0 else "tp"
                on_act = (ph1_ctr[0] % 2 == 0)
                ph1_ctr[0] += 1
                if kind == "q":
                    emit_q_group(idx, tag, on_act)
                elif kind == "k":
                    emit_k_group(idx, tag, on_act)
                else:
                    emit_vt_pair(idx, tag, on_act)

            # work scheduled before chunk-0 step jt
            ph1_step = {}
            for jt in range(NJT):
                work = []
                if jt % 4 == 0 and jt // 4 + 1 < 8:
                    work.append(("k", jt // 4 + 1))
                if jt % 2 == 0 and jt + 2 < NJT:
                    work.append(("vt", jt + 2))
                if jt in (10, 14, 18):
                    work.append(("q", (jt - 6) // 4))
                ph1_step[jt] = work

            emit_phase1("q", 0)
            emit_phase1("k", 0)
            emit_phase1("vt", 0)

            # ---- main loop ----
            pending_av = None
            pending_epi = []

            def emit_av(ex_tiles, av_views, jt):
                first = (jt == 0)
                last = (jt == NJT - 1)
                for it in range(4):
                    av = av_views[it // 2]
                    for h in range(HEADS):
                        nc.tensor.matmul(
                            av[:, it % 2, h, :],
                            lhsT=ex_tiles[h][:, it * 128:(it + 1) * 128],
                            rhs=vT_sb[:, jt, h, :],
                            start=(first and it % 2 == 0 and h == 0),
                            stop=(last and it % 2 == 1 and h == 3),
                            skip_group_check=True)

            def make_epilogue(chunk, av_views):
                co = chunk * NT
                den = workp.tile([128, 16], F32, tag="den")
                rc = workp.tile([128, 16], F32, tag="rc")
                s_t = workp.tile([128, 4, 128], BF16, tag="s")
                tp = ptp.tile([128, 512], F32, tag="tp")
                t_bf = tp[:, 0:256].bitcast(BF16)        # [128, 512] T
                pj = tp[:, 256:512]                      # [128, 256] proj psum

                def p_recip():
                    for i, av in enumerate(av_views):
                        nc.vector.tensor_copy(
                            den[:, i * 8:(i + 1) * 8],
                            av[:, :, :, 32].rearrange("p a b -> p (a b)"))
                    nc.vector.reciprocal(out=rc, in_=den)

                def p_norm(half):
                    for it in range(2 * half, 2 * half + 2):
                        av = av_views[it // 2]
                        for h in range(HEADS):
                            if half == 0:
                                nc.vector.tensor_scalar(
                                    out=s_t[:, it, h * DH:(h + 1) * DH],
                                    in0=av[:, it % 2, h, 0:DH],
                                    scalar1=rc[:, it * 4 + h:it * 4 + h + 1],
                                    scalar2=None, op0=AluOp.mult)
                            else:
                                nc.scalar.activation(
                                    s_t[:, it, h * DH:(h + 1) * DH],
                                    av[:, it % 2, h, 0:DH],
                                    mybir.ActivationFunctionType.Copy,
                                    scale=rc[:, it * 4 + h:it * 4 + h + 1])

                def p_trans():
                    for it in range(4):
                        nc.tensor.transpose(
                            t_bf[:, it * 128:(it + 1) * 128],
                            s_t[:, it, :], ident_bf)
                    nc.vector.tensor_copy(h_sb[:, co:co + NT], t_bf)

                def p_proj():
                    for it in range(4):
                        io = co + it * 128
                        nc.tensor.matmul(pj, lhsT=h_sb[:, io:io + 128],
                                         rhs=woutT_bf, start=True, stop=False)
                        nc.tensor.matmul(pj, lhsT=ones_row, rhs=bias_row,
                                         start=False, stop=True)
                        ot = outp.tile([128, C], F32, tag="out")
                        if it % 2 == 0:
                            nc.vector.tensor_copy(ot, pj)
                        else:
                            nc.scalar.activation(
                                ot, pj, mybir.ActivationFunctionType.Copy)
                        nc.sync.dma_start(out=out_d[io:io + 128, :], in_=ot)

                return [p_recip, lambda: p_norm(0), lambda: p_norm(1),
                        p_trans, p_proj]

            for chunk in range(NCHUNK):
                co = chunk * NT
                av_views = []
                for _ in range(2):
                    t = pav.tile([128, 512], F32, tag="av", name="avt")
                    av_views.append(
                        t[:, 0:264].rearrange("p (a h d) -> p a h d", a=2, h=4))
                for jt in range(NJT):
                    sims = []
                    for h in range(HEADS):
                        s = psim.tile([128, NT], F32, tag=f"s{h}")
                        nc.tensor.matmul(
                            s, lhsT=k_sb[h * DH:(h + 1) * DH,
                                         jt * JT:(jt + 1) * JT],
                            rhs=q_sb[h * DH:(h + 1) * DH, co:co + NT],
                            start=True, stop=True,
                            tile_position=(h * DH, 0))
                        sims.append(s)
                    exs = [expp.tile([128, NT], BF16, tag=f"e{h}", name=f"ex{h}")
                           for h in range(HEADS)]
                    nc.scalar.activation(exs[0], sims[0],
                                         mybir.ActivationFunctionType.Exp,
                                         scale=SCALE)
                    nc.scalar.activation(exs[1], sims[1],
                                         mybir.ActivationFunctionType.Exp,
                                         scale=SCALE)
                    nc.vector.tensor_scalar(
                        out=exs[2].bitcast(I16), in0=sims[2],
                        scalar1=EA, scalar2=EB, op0=AluOp.mult, op1=AluOp.add)
                    nc.vector.tensor_scalar(
                        out=exs[3].bitcast(I16), in0=sims[3],
                        scalar1=EA, scalar2=EB, op0=AluOp.mult, op1=AluOp.add)

                    # previous step's AV after this step's sims (PE order)
                    if pending_av is not None:
                        emit_av(*pending_av)
                    pending_av = (exs, av_views, jt)

                    if pending_epi:
                        pending_epi.pop(0)()
                    if chunk == 0:
                        for kind, idx in ph1_step.get(jt, []):
                            emit_phase1(kind, idx)

                emit_av(*pending_av)
                pending_av = None
                pending_epi = make_epilogue(chunk, av_views)
            while pending_epi:
                pending_epi.pop(0)()

    nc.compile()
    return nc


_NC = None


def _get_nc():
    global _NC
    if _NC is None:
        _NC = build_kernel()
    return _NC


def make_in_maps(x, w_qkv, w_out, b_out):
    import ml_dtypes
    x = np.ascontiguousarray(np.asarray(x, dtype=np.float32))
    w_qkv = np.asarray(w_qkv, dtype=np.float32)
    w_out = np.asarray(w_out, dtype=np.float32)
    b_out = np.asarray(b_out, dtype=np.float32)

    wqkvT = w_qkv.T                                       # [256, 384]
    woutT = w_out.T                                       # [128 hd, 256 o]

    def pack_bf16(a):
        bf = a.astype(ml_dtypes.bfloat16).view(np.uint16)
        lo = bf[:, 0::2].astype(np.uint32)
        hi = bf[:, 1::2].astype(np.uint32)
        return (lo | (hi << 16)).view(np.float32)

    bias_ones = np.zeros((128, 256 + 128), np.float32)
    bias_ones[0, 0:256] = b_out
    bias_ones[0, 256:384] = 1.0
    blob128 = np.ascontiguousarray(np.concatenate([
        pack_bf16(woutT),                                  # 128 cols
        pack_bf16(bias_ones[:, 0:256]),                    # 128 cols
        pack_bf16(np.eye(128, dtype=np.float32)),          # 64 cols
        pack_bf16(bias_ones[:, 256:384]),                  # 64 cols
    ], axis=1, dtype=np.float32))

    in_maps = []
    for core in range(8):
        b, qh = divmod(core, 2)
        xb = x[b].reshape(C, NJ)
        xrot = np.concatenate(
            [xb[:, qh * NI:(qh + 1) * NI], xb[:, (1 - qh) * NI:(2 - qh) * NI]],
            axis=1)
        blob256 = np.ascontiguousarray(np.concatenate([wqkvT, xrot], axis=1))
        in_maps.append({"blob256": blob256, "blob128": blob128})
    return in_maps


def run_spmd(x, w_qkv, w_out, b_out, **kw):
    nc = _get_nc()
    in_maps = make_in_maps(x, w_qkv, w_out, b_out)
    return run_bass_kernel_spmd(nc, in_maps, core_ids=list(range(8)), **kw)


def assemble(results):
    out = np.empty((4, C, NJ), np.float32)
    for core in range(8):
        b, qh = divmod(core, 2)
        out[b, :, qh * NI:(qh + 1) * NI] = results[core]["out_t"].T
    return out.reshape(4, C, 64, 64)


def kernel(x, w_qkv, w_out, b_out):
    res = run_spmd(x, w_qkv, w_out, b_out)
    return assemble(res.results)


# revision 10
# speedup vs baseline: 1.6739x; 1.0529x over previous
"""Trainium2 Bass kernel for nn_Attention (dense transformer spatial attention).

Reference computation (per batch b of 4):
  X = x[b] reshaped [256, 4096]                      (4096 = 64*64 pixels)
  QKV = w_qkv @ X -> [384, 4096]; q,k,v = split(QKV) each [128, 4096]
  per head h (4 heads x 32 dims): sim = (q_h*scale)^T k_h   [4096, 4096]
  attn = softmax(sim, axis=-1); out_h = attn @ v_h^T        [4096, 32]
  H = concat_heads -> [128, 4096]; out = w_out @ H + b_out  [256, 4096]

Sharding: 8 cores = (batch b in 0..3) x (query half qh in 0..1).
Each core gets full X_b (k/v need all keys) with the query half rotated
to the front (so one compiled kernel serves both qh values), computes
attention output for its 2048 queries over all 4096 keys, and the final
projection. Host gather is concatenation + transpose.

Device algorithm (per core). The kernel streams 33.5M softmax exps per
core; throughput comes from splitting the exp work across the Scalar
and Vector engines (GpSimd cannot touch PSUM) and keeping the PE stream
cost minimal:
  - inputs land as bf16 (halves the HBM stream, ~0.4% noise, well
    inside the error budget); phase 1 (q/k/vT projections) runs before
    the main loop, copies PSUM->SBUF alternating ScalarE/DVE, chasing
    the DMA pieces which are ordered query-half first.
  - sim is computed TRANSPOSED: simT[j, i] = sum_d k[d,j] q[d,i],
    row-packed K=32 matmuls (tile_position=(32h, 0)), f32r operands
    (1 cyc/row at N>=256). Heads 0/1 go to single-bank tiles s0/s1
    (ping-pong hides the refill); heads 2/3 go to one [128, 1024]
    double-buffered tile (tag sD, 2x2 banks) so DVE consumes them in a
    SINGLE tensor_scalar per step (the per-op PSUM-access init is the
    DVE tax worth amortizing).
  - exp: heads 0/1 exact on ScalarE (Exp activation, scale folded,
    bf16 out, one [128,512] activation each). Heads 2/3 via the
    Schraudolph bit-trick on DVE: bf16 exp(x*scale) ~=
    bitcast(int16(x*(128*log2e*scale) + 127*128)), one [128,1024]
    tensor_scalar writing through an int16 view. Max elem error ~6.5%
    one-sided; softmax num/denom cancellation + diffuse attention
    bring the end-to-end error to ~6e-3 of max|out| (gate is 2e-2).
    No max-subtraction (|scale*sim| < ~8).
  - AV is computed in the [i, hd] layout: av[i-tile, (h,d)] +=
    ex_h[j, i-tile]^T @ vTaug[j, d]: the exp tile slice is the
    stationary operand, so the moving operand is vT with N=33 -> 33 PE
    cycles per matmul instead of 512. vT is augmented with a ones
    column: col 32 of each head block accumulates the softmax
    denominator for free. The 16 independent 132B accumulation groups
    per bank rely on per-byte pending-zero overwrite semantics (only a
    bank's first matmul uses start=True; group check skipped).
  - psum budget (8 banks): s0 + s1 + sD(2x2) + two av banks. The
    transpose/projection tile rotates through the SAME av pool slot at
    chunk boundaries (the brief AV-matmul backlog this causes drains
    into PE's per-step slack; exp tiles are buffered deep enough that
    the ScalarE/DVE streams never notice).
  - epilogue per 512-query chunk: reciprocal of the 16 denominators +
    normalize av -> S[i, hd] bf16 (DVE), PE transpose via identity
    matmul -> T[hd, i] bf16 psum, T copied to H sbuf (ScalarE), then
    per i-tile: projection matmul lhsT=H-tile (K=128, all 4 heads at
    once, N=256 bf16) + K=1 ones-row matmul accumulating the bias,
    PSUM->SBUF copy (ScalarE), DMA out [i, o]; the host transposes.
    Slices are emitted interleaved with the next chunk's first steps
    so the engine streams never stall on the reciprocal chain.
"""

import numpy as np

import concourse.bacc as bacc
import concourse.bass as bass
import concourse.mybir as mybir
import concourse.tile as tile
from concourse.bass_utils import run_bass_kernel_spmd

F32 = mybir.dt.float32
F32R = mybir.dt.float32r
BF16 = mybir.dt.bfloat16
I16 = mybir.dt.int16

HEADS = 4
DH = 32                      # dim per head
C = 256                      # input channels
NJ = 4096                    # keys per batch (64*64)
NI = 2048                    # queries per core (half of 4096)
JT = 128                     # j tile (partition dim of simT)
NJT = NJ // JT               # 32 j tiles
NT = 512                     # i columns per step / chunk width
NCHUNK = NI // NT            # 4
SCALE = float(DH) ** -0.5
BW = 3 * 128 + NJ            # blob256 width (w | x)
# Schraudolph bf16-exp constants: bitcast(int16(x*EA + EB)) ~ exp(x*SCALE)
EA = float(SCALE * 128.0 / np.log(2.0))
EB = float(127 * 128)
AluOp = mybir.AluOpType
Act = mybir.ActivationFunctionType


def build_kernel():
    nc = bacc.Bacc("TRN2", debug=False, num_devices=8)

    # blob256 columns (bf16): [wqkvT (384) | x rotated (4096)]
    # blob128 columns: [woutT bf16 (128 f32 words) | bias-row bf16 (128) |
    #                   identity bf16 (64) | ones-row bf16 (64)]
    blob256_d = nc.dram_tensor("blob256", [C, BW], BF16, kind="ExternalInput").ap()
    blob128_d = nc.dram_tensor("blob128", [128, 384], F32, kind="ExternalInput").ap()
    out_d = nc.dram_tensor("out_t", [NI, C], F32, kind="ExternalOutput").ap()

    with tile.TileContext(nc) as tc:
        with (
            tc.tile_pool(name="singles", bufs=1) as singles,
            tc.tile_pool(name="expp", bufs=6) as expp,
            tc.tile_pool(name="workp", bufs=2) as workp,
            tc.tile_pool(name="outp", bufs=4) as outp,
            tc.tile_pool(name="psim", bufs=1, space="PSUM") as psim,
            tc.tile_pool(name="pav", bufs=2, space="PSUM") as pav,
        ):
            # ---- resident SBUF tensors ----
            blob_sb = singles.tile([128, 2, BW], BF16)    # w|x, 2 c-tiles
            w_sb = blob_sb[:, :, 0:3 * 128]
            x_sb = blob_sb[:, :, 3 * 128:BW]
            b128_sb = singles.tile([128, 384], F32)
            woutT_bf = b128_sb[:, 0:128].bitcast(BF16)    # [128 hd, 256 o]
            bias_row = b128_sb[0:1, 128:256].bitcast(BF16)   # [1, 256]
            ident_bf = b128_sb[:, 256:320].bitcast(BF16)  # [128, 128] identity
            ones_row = b128_sb[0:1, 320:384].bitcast(BF16)   # [1, 128]
            q_sb = singles.tile([128, NI], F32R)          # rows = 4h x 32d
            k_sb = singles.tile([128, NJ], F32R)
            vT_sb = singles.tile([128, NJT, HEADS, 33], BF16)  # [j, jt, h, d|1]
            h_sb = singles.tile([128, NI], BF16)          # [hd, i] normalized

            # input DMA: 512-col pieces, query-half pieces first (interleaved
            # with the other half) so q/k groups become available in the
            # order phase 1 consumes them
            PW = 512
            nc.sync.dma_start(out=blob_sb[:, 0, 0:384], in_=blob256_d[0:128, 0:384])
            nc.sync.dma_start(out=blob_sb[:, 1, 0:384], in_=blob256_d[128:256, 0:384])
            order = []
            for p in range(4):
                order.append(384 + p * PW)           # query-half piece p
                order.append(384 + NI + p * PW)      # other-half piece p
            for lo in order:
                for ct in range(2):
                    nc.sync.dma_start(
                        out=blob_sb[:, ct, lo:lo + PW],
                        in_=blob256_d[ct * 128:(ct + 1) * 128, lo:lo + PW])
            nc.sync.dma_start(out=b128_sb, in_=blob128_d)

            # ones column of vT (col 32 of each (jt, h) block)
            nc.gpsimd.memset(vT_sb[:, :, :, 32], 1.0)

            # warm the ScalarE exp table during the DMA wait
            warm = singles.tile([1, 1], F32)
            nc.gpsimd.memset(warm, 0.0)
            nc.scalar.activation(warm, warm, Act.Exp)

            # ---- phase 1: q/k/vT projections (serial, DMA-paced) ----
            # psum rotates through the sim/av tags (all free before the main
            # loop); copies alternate ScalarE / DVE
            ph1_ctr = [0]

            def ph1_psum(name):
                i = ph1_ctr[0] % 4
                ph1_ctr[0] += 1
                if i < 2:
                    return psim.tile([128, NT], F32, tag=f"s{i}", name=name)
                if i == 2:
                    return psim.tile([128, 2 * NT], F32, tag="sD",
                                     bufs=2, name=name)[:, 0:NT]
                return pav.tile([128, NT], F32, tag="av", name=name)

            def ph1_copy(dst, src, on_act):
                if on_act:
                    nc.scalar.activation(dst, src, Act.Copy)
                else:
                    nc.vector.tensor_copy(dst, src)

            def emit_q_group(g, on_act):
                ps = ph1_psum("ph1q")
                for ct in range(2):
                    nc.tensor.matmul(
                        ps, lhsT=w_sb[:, ct, 0:128],
                        rhs=x_sb[:, ct, g * NT:(g + 1) * NT],
                        start=(ct == 0), stop=(ct == 1))
                ph1_copy(q_sb[:, g * NT:(g + 1) * NT], ps, on_act)

            def emit_k_group(g, on_act):
                ps = ph1_psum("ph1k")
                for ct in range(2):
                    nc.tensor.matmul(
                        ps, lhsT=w_sb[:, ct, 128:256],
                        rhs=x_sb[:, ct, g * NT:(g + 1) * NT],
                        start=(ct == 0), stop=(ct == 1))
                ph1_copy(k_sb[:, g * NT:(g + 1) * NT], ps, on_act)

            def emit_vt_pair(jt, on_act):
                ps = ph1_psum("ph1v")
                for half in range(2):
                    o = half * 128
                    nc.tensor.matmul(
                        ps[:, o:o + 128],
                        lhsT=x_sb[:, 0, (jt + half) * JT:(jt + half + 1) * JT],
                        rhs=w_sb[:, 0, 256:384], start=True, stop=False)
                    nc.tensor.matmul(
                        ps[:, o:o + 128],
                        lhsT=x_sb[:, 1, (jt + half) * JT:(jt + half + 1) * JT],
                        rhs=w_sb[:, 1, 256:384], start=False, stop=True)
                ph1_copy(vT_sb[:, jt:jt + 2, :, 0:DH],
                         ps[:, 0:256].rearrange("p (a h d) -> p a h d", a=2, h=4),
                         on_act)

            ph1 = [("q", 0)]
            for g in range(8):
                ph1.append(("k", g))
                ph1.append(("vt", 4 * g))
                ph1.append(("vt", 4 * g + 2))
                if g in (1, 3, 5):
                    ph1.append(("q", (g + 1) // 2))
            for i, (kind, idx) in enumerate(ph1):
                on_act = (i % 2 == 0)
                if kind == "q":
                    emit_q_group(idx, on_act)
                elif kind == "k":
                    emit_k_group(idx, on_act)
                else:
                    emit_vt_pair(idx, on_act)

            # ---- main loop ----
            pending_av = None
            pending_epi = []

            def emit_av(ex_of_h, av_views, jt):
                first = (jt == 0)
                last = (jt == NJT - 1)
                for it in range(4):
                    av = av_views[it // 2]
                    for h in range(HEADS):
                        nc.tensor.matmul(
                            av[:, it % 2, h, :],
                            lhsT=ex_of_h(h, it),
                            rhs=vT_sb[:, jt, h, :],
                            start=(first and it % 2 == 0 and h == 0),
                            stop=(last and it % 2 == 1 and h == 3),
                            skip_group_check=True)

            def make_epilogue(chunk, av_views):
                co = chunk * NT
                den = workp.tile([128, 16], F32, tag="den")
                rc = workp.tile([128, 16], F32, tag="rc")
                s_t = workp.tile([128, 4, 128], BF16, tag="s")
                tp = pav.tile([128, 512], F32, tag="av", name="tp")
                t_bf = tp[:, 0:256].bitcast(BF16)        # [128, 512] T
                pj = tp[:, 256:512]                      # [128, 256] proj psum

                def p_recip():
                    for i, av in enumerate(av_views):
                        nc.vector.tensor_copy(
                            den[:, i * 8:(i + 1) * 8],
                            av[:, :, :, 32].rearrange("p a b -> p (a b)"))
                    nc.vector.reciprocal(out=rc, in_=den)

                def p_norm(half):
                    for it in range(2 * half, 2 * half + 2):
                        av = av_views[it // 2]
                        for h in range(HEADS):
                            nc.vector.tensor_scalar(
                                out=s_t[:, it, h * DH:(h + 1) * DH],
                                in0=av[:, it % 2, h, 0:DH],
                                scalar1=rc[:, it * 4 + h:it * 4 + h + 1],
                                scalar2=None, op0=AluOp.mult)

                def p_trans():
                    for it in range(4):
                        nc.tensor.transpose(
                            t_bf[:, it * 128:(it + 1) * 128],
                            s_t[:, it, :], ident_bf)
                    nc.scalar.activation(h_sb[:, co:co + NT], t_bf, Act.Copy)

                def p_proj():
                    for it in range(4):
                        io = co + it * 128
                        nc.tensor.matmul(pj, lhsT=h_sb[:, io:io + 128],
                                         rhs=woutT_bf, start=True, stop=False)
                        nc.tensor.matmul(pj, lhsT=ones_row, rhs=bias_row,
                                         start=False, stop=True)
                        ot = outp.tile([128, C], F32, tag="out")
                        nc.scalar.activation(ot, pj, Act.Copy)
                        nc.sync.dma_start(out=out_d[io:io + 128, :], in_=ot)

                return [p_recip, lambda: p_norm(0), lambda: p_norm(1),
                        p_trans, p_proj]

            for chunk in range(NCHUNK):
                co = chunk * NT
                av_views = []
                for nm in ("avA", "avB"):
                    t = pav.tile([128, 512], F32, tag="av", name=nm)
                    av_views.append(
                        t[:, 0:264].rearrange("p (a h d) -> p a h d", a=2, h=4))
                for jt in range(NJT):
                    s0 = psim.tile([128, NT], F32, tag="s0")
                    s1 = psim.tile([128, NT], F32, tag="s1")
                    sD = psim.tile([128, 2 * NT], F32, tag="sD", bufs=2)
                    targets = (s0, s1, sD[:, 0:NT], sD[:, NT:2 * NT])
                    for h in range(HEADS):
                        nc.tensor.matmul(
                            targets[h],
                            lhsT=k_sb[h * DH:(h + 1) * DH,
                                      jt * JT:(jt + 1) * JT],
                            rhs=q_sb[h * DH:(h + 1) * DH, co:co + NT],
                            start=True, stop=True,
                            tile_position=(h * DH, 0))
                    e0 = expp.tile([128, NT], BF16, tag="e0")
                    e1 = expp.tile([128, NT], BF16, tag="e1")
                    e23 = expp.tile([128, 2 * NT], BF16, tag="e23")
                    nc.scalar.activation(e0, s0, Act.Exp, scale=SCALE)
                    nc.scalar.activation(e1, s1, Act.Exp, scale=SCALE)
                    nc.vector.tensor_scalar(
                        out=e23.bitcast(I16), in0=sD,
                        scalar1=EA, scalar2=EB, op0=AluOp.mult, op1=AluOp.add)

                    def ex_of_h(h, it, e0=e0, e1=e1, e23=e23):
                        if h == 0:
                            return e0[:, it * 128:(it + 1) * 128]
                        if h == 1:
                            return e1[:, it * 128:(it + 1) * 128]
                        off = (h - 2) * NT + it * 128
                        return e23[:, off:off + 128]

                    if pending_av is not None:
                        emit_av(*pending_av)
                    pending_av = (ex_of_h, av_views, jt)

                    if pending_epi:
                        pending_epi.pop(0)()

                emit_av(*pending_av)
                pending_av = None
                pending_epi = make_epilogue(chunk, av_views)
            while pending_epi:
                pending_epi.pop(0)()

    nc.compile()
    return nc


_NC = None


def _get_nc():
    global _NC
    if _NC is None:
        _NC = build_kernel()
    return _NC


def make_in_maps(x, w_qkv, w_out, b_out):
    import ml_dtypes
    x = np.ascontiguousarray(np.asarray(x, dtype=np.float32))
    w_qkv = np.asarray(w_qkv, dtype=np.float32)
    w_out = np.asarray(w_out, dtype=np.float32)
    b_out = np.asarray(b_out, dtype=np.float32)

    wqkvT = w_qkv.T                                       # [256, 384]
    woutT = w_out.T                                       # [128 hd, 256 o]

    def pack_bf16(a):
        bf = a.astype(ml_dtypes.bfloat16).view(np.uint16)
        lo = bf[:, 0::2].astype(np.uint32)
        hi = bf[:, 1::2].astype(np.uint32)
        return (lo | (hi << 16)).view(np.float32)

    bias_ones = np.zeros((128, 256 + 128), np.float32)
    bias_ones[0, 0:256] = b_out
    bias_ones[0, 256:384] = 1.0
    blob128 = np.ascontiguousarray(np.concatenate([
        pack_bf16(woutT),                                  # 128 cols
        pack_bf16(bias_ones[:, 0:256]),                    # 128 cols
        pack_bf16(np.eye(128, dtype=np.float32)),          # 64 cols
        pack_bf16(bias_ones[:, 256:384]),                  # 64 cols
    ], axis=1, dtype=np.float32))

    in_maps = []
    for core in range(8):
        b, qh = divmod(core, 2)
        xb = x[b].reshape(C, NJ)
        xrot = np.concatenate(
            [xb[:, qh * NI:(qh + 1) * NI], xb[:, (1 - qh) * NI:(2 - qh) * NI]],
            axis=1)
        blob256 = np.concatenate([wqkvT, xrot], axis=1).astype(ml_dtypes.bfloat16)
        in_maps.append({"blob256": np.ascontiguousarray(blob256),
                        "blob128": blob128})
    return in_maps


def run_spmd(x, w_qkv, w_out, b_out, **kw):
    nc = _get_nc()
    in_maps = make_in_maps(x, w_qkv, w_out, b_out)
    return run_bass_kernel_spmd(nc, in_maps, core_ids=list(range(8)), **kw)


def assemble(results):
    out = np.empty((4, C, NJ), np.float32)
    for core in range(8):
        b, qh = divmod(core, 2)
        out[b, :, qh * NI:(qh + 1) * NI] = results[core]["out_t"].T
    return out.reshape(4, C, 64, 64)


def kernel(x, w_qkv, w_out, b_out):
    res = run_spmd(x, w_qkv, w_out, b_out)
    return assemble(res.results)
